# revision 31
# baseline (speedup 1.0000x reference)
"""DBRX-style MoE (E=16, top-4, C=2048, H=3584, N=1024 tokens) on 8 TRN2 cores.

Strategy (expert-parallel x 2-way FFN sharding, routed):
  - Host: gating in fp64 (logits -> top-4 -> softmax weights). fp64 makes the
    selected expert SET maximally robust against fp rounding (min 4th/5th logit
    gap in the data is ~1e-6; on-device fp32 selection could flip experts vs
    the fp32 reference).
  - Each expert is split into 2 shards along the FFN dim (H/2 = 1792 rows of
    W_up/W_gate, matching W_down columns); the 32 shards are dealt to
    8 cores x 4 slots by descending token count, so the SPMD per-slot caps
    (max count within each rank octile) pad only ~4% above the ideal balance
    (vs ~9% for whole-expert slots).
  - Device (per core, per slot): uT/gT = Wup/Wg @ xT (PSUM-accumulated over
    C chunks, fp16 matmuls, exact fp32 accumulation), hT = silu(gT) * uT *
    gate_weight, then yT = Wdown @ hT accumulated over the shard's 14 H
    chunks; yT stored fp16.
  - Host: scatter-add each shard's yT columns back to its token rows (the
    two H-halves of an expert sum there).
  - Schedule: first slot's x transfer split in four, interleaved with the
    first weight tiles, so the PE starts ~2us after the engine-init preamble;
    weights stream at ~90% DMA duty behind the PE.

Padding slots have gate weight 0 and their yT columns are never read back.
"""

import math

import numpy as np

E, TOPK = 16, 4
C, H = 2048, 3584
B, T = 2, 512
N = B * T
N_CORES = 8
HSPLIT = 2                      # FFN-dim shards per expert
N_SLOTS = E * HSPLIT // N_CORES  # 4 slots per core
H_PER = H // HSPLIT             # 1792
C_CHUNKS = C // 128             # 16
H_CHUNKS = H_PER // 128         # 14

YT_F16 = True       # store y back to HBM in fp16 (err +~1e-4, halves y DMA)
TAIL_SPLIT = True   # split the last ct group in halves to overlap the final store
WARMUP_MM = 12      # 512-col dummy PE matmuls during the DMA fill window
WARMUP_SMALL = 3    # trailing 128-col dummies for a granular finish

_NC_CACHE: dict[tuple, object] = {}


def _token_tiles(cap: int) -> list[tuple[int, int]]:
    """Split [0, cap) into free-dim tiles of at most 512 (PSUM bank limit)."""
    tiles = []
    off = 0
    while off < cap:
        sz = min(512, cap - off)
        tiles.append((off, sz))
        off += sz
    return tiles


def _build_nc(caps: tuple):
    import concourse.bacc as bacc
    import concourse.mybir as mybir
    import concourse.tile as tile

    f32 = mybir.dt.float32
    f16 = mybir.dt.float16
    yt_dt = f16 if YT_F16 else f32

    nc = bacc.Bacc("TRN2", target_bir_lowering=False, debug=False)
    xgs = [
        nc.dram_tensor(f"xg{j}", [128, C_CHUNKS * caps[j]], f16, kind="ExternalInput")
        for j in range(N_SLOTS)
    ]
    wbs = [
        nc.dram_tensor(f"wb{j}", [128, caps[j]], f32, kind="ExternalInput")
        for j in range(N_SLOTS)
    ]
    # up||gate packed per h-chunk: 8KB per-partition DMA descriptors instead
    # of 2x4KB (per-descriptor turnaround is ~10-15% of queue time at 4KB).
    wug = nc.dram_tensor(
        "wug", [N_SLOTS, H_CHUNKS, 128, 2 * C_CHUNKS * 128], f16, kind="ExternalInput"
    )
    # W_down ct-chunks paired: 7KB descriptors.
    wd = nc.dram_tensor(
        "wd", [N_SLOTS, C_CHUNKS // 2, 128, 2 * H_CHUNKS * 128], f16, kind="ExternalInput"
    )
    # y stored 4 ct-chunks per DMA: [grp, q, k, t] with c = (4*grp+k)*128+q.
    yts = [
        nc.dram_tensor(
            f"yt{j}", [C_CHUNKS // 4, 128, 4 * caps[j]], yt_dt, kind="ExternalOutput"
        )
        for j in range(N_SLOTS)
    ]

    with tile.TileContext(nc) as tc:
        with (
            tc.tile_pool(name="xp", bufs=2) as xp,
            tc.tile_pool(name="wp", bufs=11) as wp,
            tc.tile_pool(name="hp", bufs=2) as hp,
            tc.tile_pool(name="wdp", bufs=6) as wdp,
            tc.tile_pool(name="sp", bufs=3) as sp,
            tc.tile_pool(name="psu", bufs=2, space="PSUM") as psu,
            tc.tile_pool(name="psg", bufs=2, space="PSUM") as psg,
            tc.tile_pool(name="psy", bufs=2, space="PSUM") as psy,
            tc.tile_pool(name="psw", bufs=1, space="PSUM") as psw,
        ):
            if WARMUP_MM:
                # Run the PE on throwaway matmuls while the first x/weight
                # tiles stream in: the p-state ramp (half clock for the first
                # ~5-8us of execution) completes before real work arrives.
                # Sized to end just as the head DMA completes (~12.5us).
                wz = sp.tile([128, 512], f16, tag="warmz")
                nc.vector.memset(wz[:], 0.0)
                wps = psw.tile([128, 512], f32, tag="warmp")
                for i in range(WARMUP_MM + WARMUP_SMALL):
                    cols = 512 if i < WARMUP_MM else 128
                    nc.tensor.matmul(
                        wps[:, :cols],
                        wz[:, :128],
                        wz[:, :cols],
                        start=True,
                        stop=True,
                    )
            # Largest slot first: its L1 phase has the most PE time per weight
            # byte, so the DMA builds up a buffer lead for the tighter small
            # slots at the end (which also shortens the tail groups).
            slot_order = sorted(range(N_SLOTS), key=lambda j: -caps[j])
            for si, e in enumerate(slot_order):
                cap = caps[e]
                tts = _token_tiles(cap)
                xt = xp.tile([128, C_CHUNKS * cap], f16, tag="xg")
                wbt = xp.tile([128, cap], f32, tag="wb")
                head0 = False
                if si == 0:
                    # Head: the first matmul only needs x c-chunk 0 and the up
                    # half of the first packed weight tile. Interleave the xg
                    # transfer (split in 4) with the two halves of wupg0 so
                    # the PE starts streaming as soon as possible.
                    # Order: x q0, up-half, x q1-q3, wb, gate-half — the
                    # u-group (first 16 matmuls) needs up + ALL x quarters;
                    # the gate half is only needed ~4us later.
                    qc = C_CHUNKS // 4
                    nc.sync.dma_start(
                        xt[:, : qc * cap], xgs[e].ap()[:, : qc * cap]
                    )
                    wpg0 = wp.tile([128, 2 * C_CHUNKS * 128], f16, tag="wug")
                    nc.sync.dma_start(
                        wpg0[:, : C_CHUNKS * 128], wug.ap()[e, 0][:, : C_CHUNKS * 128]
                    )
                    nc.sync.dma_start(
                        xt[:, qc * cap : 2 * qc * cap],
                        xgs[e].ap()[:, qc * cap : 2 * qc * cap],
                    )
                    nc.sync.dma_start(
                        xt[:, 2 * qc * cap : 3 * qc * cap],
                        xgs[e].ap()[:, 2 * qc * cap : 3 * qc * cap],
                    )
                    nc.sync.dma_start(
                        xt[:, 3 * qc * cap :], xgs[e].ap()[:, 3 * qc * cap :]
                    )
                    nc.sync.dma_start(wbt[:], wbs[e].ap())
                    nc.sync.dma_start(
                        wpg0[:, C_CHUNKS * 128 :], wug.ap()[e, 0][:, C_CHUNKS * 128 :]
                    )
                    head0 = True
                else:
                    nc.sync.dma_start(xt[:], xgs[e].ap())
                    nc.sync.dma_start(wbt[:], wbs[e].ap())
                ht = hp.tile([128, H_CHUNKS * cap], f16, tag="ht")

                for h in range(H_CHUNKS):
                    if h == 0 and head0:
                        wpg = wpg0
                    else:
                        wpg = wp.tile([128, 2 * C_CHUNKS * 128], f16, tag="wug")
                        nc.sync.dma_start(wpg[:], wug.ap()[e, h])
                    goff = C_CHUNKS * 128
                    for off, sz in tts:
                        ups = psu.tile([128, sz], f32, tag="u")
                        gps = psg.tile([128, sz], f32, tag="g")
                        for c in range(C_CHUNKS):
                            nc.tensor.matmul(
                                ups[:],
                                wpg[:, c * 128 : (c + 1) * 128],
                                xt[:, c * cap + off : c * cap + off + sz],
                                start=(c == 0),
                                stop=(c == C_CHUNKS - 1),
                            )
                        for c in range(C_CHUNKS):
                            nc.tensor.matmul(
                                gps[:],
                                wpg[:, goff + c * 128 : goff + (c + 1) * 128],
                                xt[:, c * cap + off : c * cap + off + sz],
                                start=(c == 0),
                                stop=(c == C_CHUNKS - 1),
                            )
                        sg = sp.tile([128, cap], f32, tag="sg")
                        nc.scalar.activation(
                            sg[:, :sz], gps[:], mybir.ActivationFunctionType.Silu
                        )
                        uw = sp.tile([128, cap], f32, tag="uw")
                        nc.vector.tensor_mul(
                            uw[:, :sz], ups[:], wbt[:, off : off + sz]
                        )
                        nc.vector.tensor_mul(
                            ht[:, h * cap + off : h * cap + off + sz],
                            sg[:, :sz],
                            uw[:, :sz],
                        )

                last_slot = si == N_SLOTS - 1
                yo4 = None
                for ct in range(C_CHUNKS):
                    if ct % 2 == 0:
                        wdt = wdp.tile([128, 2 * H_CHUNKS * 128], f16, tag="wd")
                        nc.sync.dma_start(wdt[:], wd.ap()[e, ct // 2])
                    wbase = (ct % 2) * H_CHUNKS * 128
                    if ct % 4 == 0:
                        yo4 = sp.tile([128, 4 * cap], yt_dt, tag="yo")
                    k = ct % 4
                    tts_ct = tts
                    if (
                        TAIL_SPLIT
                        and last_slot
                        and ct == C_CHUNKS - 1
                        and len(tts) == 1
                    ):
                        # Halve the very last PSUM group so the first half's
                        # copy+store overlaps the second half's matmuls.
                        half = (cap // 8) * 4
                        tts_ct = [(0, half), (half, cap - half)]
                    for off, sz in tts_ct:
                        yps = psy.tile([128, sz], f32, tag="y")
                        for h in range(H_CHUNKS):
                            nc.tensor.matmul(
                                yps[:],
                                wdt[:, wbase + h * 128 : wbase + (h + 1) * 128],
                                ht[:, h * cap + off : h * cap + off + sz],
                                start=(h == 0),
                                stop=(h == H_CHUNKS - 1),
                            )
                        nc.vector.tensor_copy(
                            yo4[:, k * cap + off : k * cap + off + sz], yps[:]
                        )
                        if last_slot and ct >= C_CHUNKS - 4:
                            # Tail: store per-(sub)tile so the final DMA is
                            # small and overlaps the remaining matmuls.
                            nc.sync.dma_start(
                                yts[e].ap()[ct // 4][:, k * cap + off : k * cap + off + sz],
                                yo4[:, k * cap + off : k * cap + off + sz],
                            )
                    if ct % 4 == 3 and not (last_slot and ct == C_CHUNKS - 1):
                        nc.sync.dma_start(yts[e].ap()[ct // 4], yo4[:])
    nc.compile()
    return nc


def _get_nc(caps: tuple):
    if caps not in _NC_CACHE:
        _NC_CACHE[caps] = _build_nc(caps)
    return _NC_CACHE[caps]


def _route(xf: np.ndarray, gate_inp: np.ndarray):
    """Host gating in fp64: per-expert token index lists + combine weights."""
    logits = xf.astype(np.float64) @ gate_inp.astype(np.float64).T  # [N, E]
    # top-4 (descending); fp64 makes ordering robust vs the fp32 reference
    topi = np.argsort(-logits, axis=1, kind="stable")[:, :TOPK]  # [N, K]
    topv = np.take_along_axis(logits, topi, axis=1)
    w = np.exp(topv - topv[:, :1])
    w /= w.sum(axis=1, keepdims=True)  # [N, K] fp64 softmax
    idxs, wts = [], []
    for e in range(E):
        sel = topi == e  # [N, K]
        rows = np.nonzero(sel.any(axis=1))[0]
        k_of_row = np.argmax(sel[rows], axis=1)  # which top-k slot holds e
        idxs.append(rows.astype(np.int64))
        wts.append(w[rows, k_of_row])
    return idxs, wts


def kernel(x, W_up, W_gate, W_down, gate_inp):
    from concourse import bass_utils

    x = np.ascontiguousarray(np.asarray(x, dtype=np.float32))
    W_up = np.asarray(W_up, dtype=np.float32)
    W_gate = np.asarray(W_gate, dtype=np.float32)
    W_down = np.asarray(W_down, dtype=np.float32)
    gate_inp = np.asarray(gate_inp, dtype=np.float32)

    xf = x.reshape(N, C)
    idxs, wts = _route(xf, gate_inp)
    counts = np.array([len(i) for i in idxs])
    # 32 shards = (expert, H-half); shard token count = expert count. Sort by
    # count desc; slot j of every core gets rank block j, so caps pad only
    # within a rank octile (SPMD needs equal shapes across cores).
    shards = [(e, hh) for e in range(E) for hh in range(HSPLIT)]
    order = sorted(range(len(shards)), key=lambda s: -counts[shards[s][0]])
    assign = [
        [shards[order[j * N_CORES + core]] for j in range(N_SLOTS)]
        for core in range(N_CORES)
    ]
    caps = tuple(
        max(
            64,
            int(
                math.ceil(
                    max(counts[shards[order[j * N_CORES + c]][0]] for c in range(N_CORES)) / 4
                )
            )
            * 4,
        )
        for j in range(N_SLOTS)
    )

    in_maps = []
    xg_cache: dict[tuple[int, int], np.ndarray] = {}
    for core in range(N_CORES):
        wug = np.empty((N_SLOTS, H_CHUNKS, 128, 2 * C_CHUNKS * 128), np.float16)
        wdl = np.empty((N_SLOTS, C_CHUNKS // 2, 128, 2 * H_CHUNKS * 128), np.float16)
        im = {"wug": wug, "wd": wdl}
        for j in range(N_SLOTS):
            cap = caps[j]
            e, hh = assign[core][j]
            hb = hh * H_PER
            idx, wvec = idxs[e], wts[e]
            cnt = len(idx)
            key = (e, cap)
            if key not in xg_cache:
                xge = np.zeros((cap, C), np.float16)
                xge[:cnt] = xf[idx]
                # [q, c_chunk, t] <- xge[t, c_chunk*128+q]
                xg_cache[key] = (
                    xge.reshape(cap, C_CHUNKS, 128)
                    .transpose(2, 1, 0)
                    .reshape(128, C_CHUNKS * cap)
                )
            wbl = np.zeros((128, cap), np.float32)
            wbl[:, :cnt] = np.float32(wvec)[None, :]
            # stationary tiles, up||gate packed: [h_chunk, q(c_in), c_chunk, hcol]
            wug[j, :, :, : C_CHUNKS * 128] = (
                W_up[e, hb : hb + H_PER]
                .reshape(H_CHUNKS, 128, C_CHUNKS, 128)
                .transpose(0, 3, 2, 1)
                .reshape(H_CHUNKS, 128, C_CHUNKS * 128)
            )
            wug[j, :, :, C_CHUNKS * 128 :] = (
                W_gate[e, hb : hb + H_PER]
                .reshape(H_CHUNKS, 128, C_CHUNKS, 128)
                .transpose(0, 3, 2, 1)
                .reshape(H_CHUNKS, 128, C_CHUNKS * 128)
            )
            # [c_tile, q(h_in), h_chunk, ccol], then ct-chunks paired per row
            wdo = (
                W_down[e, :, hb : hb + H_PER]
                .reshape(C_CHUNKS, 128, H_CHUNKS, 128)
                .transpose(0, 3, 2, 1)
                .reshape(C_CHUNKS, 128, H_CHUNKS * 128)
            )
            wdl[j] = (
                wdo.reshape(C_CHUNKS // 2, 2, 128, H_CHUNKS * 128)
                .transpose(0, 2, 1, 3)
                .reshape(C_CHUNKS // 2, 128, 2 * H_CHUNKS * 128)
            )
            im[f"xg{j}"] = xg_cache[key]
            im[f"wb{j}"] = wbl
        in_maps.append(im)

    nc = _get_nc(caps)
    res = bass_utils.run_bass_kernel_spmd(nc, in_maps, core_ids=list(range(N_CORES)))
    kernel.last_result = res

    y = np.zeros((N, C), np.float32)
    for core in range(N_CORES):
        for j in range(N_SLOTS):
            e, hh = assign[core][j]
            idx = idxs[e]
            cnt = len(idx)
            cap = caps[j]
            ytf = (
                res.results[core][f"yt{j}"]
                .reshape(C_CHUNKS // 4, 128, 4, cap)
                .transpose(0, 2, 1, 3)
                .reshape(C, cap)
            )
            y[idx] += ytf[:, :cnt].T.astype(np.float32)
    return y.reshape(B, T, C)
